# revision 28
# baseline (speedup 1.0000x reference)
"""GA3 Conv2d kernel for 8 Trainium2 NeuronCores.

Math: the reference computes, per batch image,
    out[b, co, m] = sum_{j,k} S[m,j,k] * (conv2d(a_k, W[j]) + bias[j])[co]
with a_k = x[:, k::8] (blade-interleaved channels).  Because the sign
combination is linear, it folds into the conv weights:
    V[co*8+m, ci*8+k, kh, kw] = sum_j S[m,j,k] * W[j, co, ci, kh, kw]
    bias_eff[co*8+m]          = sum_{j,k} S[m,j,k] * b[j, co]
so the whole module is ONE dense 3x3 conv with Cin=Cout=128 on
[B, 128, 128, 128].  We shard data-parallel over B across the 8 cores
(1 image per core) and implement the conv as 9 shifted float32r matmuls
per 4-row output block (tap weights stationary [ic=128 x oc=128], pixels
moving, fp32 PSUM accumulation).

Layout: the host pre-pads each image into a flat per-partition buffer
    [pad pair][row: 128 data + pad pair] x 130 padded rows
(pitch 130, zeros at the halo) so every device-side load is a fully
contiguous DMA and tap shifts are pure address offsets.  Input rows
stream in on the SP HWDGE ring in 9 chunks while weights+bias (packed
into one [128, 9*128+1] tensor) and output stores use the ACT ring.
Dummy warm-up matmuls run during the head DMAs to lift the PE HAM clock
gate to 2.4 GHz before the real work starts.
"""

import numpy as np

_TERMS = [
    [(0, 0, 1), (1, 1, 1), (2, 2, 1), (3, 3, 1), (4, 4, -1), (5, 5, -1), (6, 6, -1), (7, 7, -1)],
    [(1, 0, 1), (0, 1, 1), (2, 4, 1), (4, 2, -1), (3, 6, 1), (6, 3, -1), (5, 7, -1), (7, 5, -1)],
    [(2, 0, 1), (0, 2, 1), (1, 4, -1), (4, 1, 1), (3, 5, 1), (5, 3, -1), (6, 7, 1), (7, 6, 1)],
    [(3, 0, 1), (0, 3, 1), (1, 6, -1), (6, 1, 1), (2, 5, -1), (5, 2, 1), (4, 7, -1), (7, 4, -1)],
    [(4, 0, 1), (0, 4, 1), (2, 1, 1), (1, 2, -1), (3, 7, 1), (7, 3, 1), (6, 5, 1), (5, 6, -1)],
    [(5, 0, 1), (0, 5, 1), (3, 2, 1), (2, 3, -1), (1, 7, 1), (7, 1, 1), (4, 6, 1), (6, 4, -1)],
    [(6, 0, 1), (0, 6, 1), (3, 1, 1), (1, 3, -1), (2, 7, -1), (7, 2, -1), (5, 4, 1), (4, 5, -1)],
    [(7, 0, 1), (0, 7, 1), (5, 1, 1), (1, 5, 1), (6, 2, -1), (2, 6, -1), (4, 3, 1), (3, 4, 1)],
]
_S = np.zeros((8, 8, 8), dtype=np.float32)
for _m, _terms in enumerate(_TERMS):
    for _j, _k, _s in _terms:
        _S[_m, _j, _k] = _s

B, CIN, COUT, H, W = 8, 16, 16, 128, 128
C = 8 * CIN  # 128 interleaved channels
N_CORES = 8
STRIP = 16          # output rows per strip (one store DMA)
N_STRIPS = H // STRIP
GROUP = 4           # output rows per PSUM accumulation group (4*128 = 512 free)
PW = W + 2          # padded row pitch in the flat layout
NPR = H + 2         # padded rows (-1 .. 128)
FLAT = 2 + NPR * PW  # flat elems/partition: leading pad pair + 130 rows
WCOLS = 9 * C + 1   # packed weight taps + bias column
N_WARMUP = 8        # HAM warm-up matmuls during the head DMAs

_CACHED_NC = None


def _build_nc():
    import concourse.bass as bass
    import concourse.mybir as mybir
    import concourse.tile as tile
    from concourse import bacc

    f32 = mybir.dt.float32
    f32r = mybir.dt.float32r

    nc = bacc.Bacc("TRN2", target_bir_lowering=False, debug=False,
                   enable_asserts=False)

    xb = nc.dram_tensor("xb", [C, FLAT], f32r, kind="ExternalInput").ap()
    wf = nc.dram_tensor("wf", [C, WCOLS], f32r, kind="ExternalInput").ap()
    out = nc.dram_tensor("out", [C, H, W], f32, kind="ExternalOutput").ap()

    with tile.TileContext(nc) as tc:
        with (
            tc.tile_pool(name="wpool", bufs=1) as wpool,
            tc.tile_pool(name="xpool", bufs=1) as xpool,
            tc.tile_pool(name="pspool", bufs=8, space="PSUM") as pspool,
            tc.tile_pool(name="opool", bufs=3) as opool,
        ):
            xfull = xpool.tile([C, FLAT], f32r)
            wtile = wpool.tile([C, WCOLS], f32r)

            # weights split so early taps unblock the first matmuls sooner
            nc.scalar.dma_start(out=wtile[:, 0:2 * C], in_=wf[:, 0:2 * C])
            nc.scalar.dma_start(out=wtile[:, 2 * C:WCOLS],
                                in_=wf[:, 2 * C:WCOLS])
            btile = wtile[:, 9 * C:WCOLS].bitcast(f32)

            # input chunks (contiguous flat ranges at padded-row boundaries);
            # first chunk halved so the first PSUM group starts early.  Only
            # the first three are issued at the head — the rest are emitted
            # interleaved with the strips so head DMAs don't collide on the
            # 8 shared DMA-completion semaphore lanes.
            bounds = [0, 6, 9, 25, 41, 57, 73, 89, 105, 121, NPR]

            def emit_chunk(c):
                a, b = bounds[c], bounds[c + 1]
                lo = 0 if a == 0 else 2 + PW * a
                hi = 2 + PW * b if b < NPR else FLAT
                nc.sync.dma_start(out=xfull[:, lo:hi], in_=xb[:, lo:hi])

            for c in range(5):
                emit_chunk(c)

            # ---- conv: 32 PSUM groups x 9 shifted matmuls
            for s in range(N_STRIPS):
                if 2 <= s <= 6:
                    emit_chunk(s + 3)   # stays ~2 strips ahead of consumption
                obuf = opool.tile([C, STRIP * W], f32)
                for g in range(STRIP // GROUP):
                    ps = pspool.tile([C, GROUP * W], f32)
                    ta = 0
                    for dh in range(3):
                        for dw in range(3):
                            # out rows R..R+3 read padded rows R+dh..R+dh+3
                            # at column shift dw-1; padded row pr's data
                            # starts at flat offset 2 + 130*pr
                            base = 1 + PW * (16 * s + 4 * g + dh) + dw
                            rhs = bass.AP(xfull.tensor, xfull.offset + base,
                                          [xfull.ap[0], [PW, GROUP], [1, W]])
                            nc.tensor.matmul(
                                ps[:, :],
                                lhsT=wtile[:, ta * C:(ta + 1) * C],
                                rhs=rhs,
                                start=(ta == 0),
                                stop=(ta == 8),
                            )
                            ta += 1
                    nc.vector.tensor_scalar_add(
                        out=obuf[:, g * GROUP * W:(g + 1) * GROUP * W],
                        in0=ps[:, :],
                        scalar1=btile,
                    )
                if s < N_STRIPS - 1:
                    nc.scalar.dma_start(
                        out=out[:, s * STRIP:(s + 1) * STRIP, :],
                        in_=obuf[:, :])
                else:
                    # split the final store so the tail barrier waits on a
                    # small last transfer
                    for g in range(STRIP // GROUP):
                        nc.scalar.dma_start(
                            out=out[:, s * STRIP + g * GROUP:
                                    s * STRIP + (g + 1) * GROUP, :],
                            in_=obuf[:, g * GROUP * W:(g + 1) * GROUP * W])

    nc.compile()
    return nc


def _get_nc():
    global _CACHED_NC
    if _CACHED_NC is None:
        _CACHED_NC = _build_nc()
    return _CACHED_NC


def _prep_weights(Wfull: np.ndarray, b: np.ndarray):
    # wf[ic, tap*128 + oc] with ic = ci*8+k, oc = co*8+m, tap = kh*3+kw;
    # final column (index 9*128) holds bias_eff[oc] indexed by partition.
    V = np.einsum("mjk,jcihw->ikhwcm", _S.astype(np.float64),
                  Wfull.astype(np.float64))          # [ci,k,kh,kw,co,m]
    V = V.reshape(C, 9 * C)
    bias = np.einsum("mjk,jc->cm", _S.astype(np.float64),
                     b.astype(np.float64)).reshape(C, 1)
    wf = np.concatenate([V, bias], axis=1)
    return np.ascontiguousarray(wf, dtype=np.float32)


def _pad_images(x: np.ndarray) -> np.ndarray:
    # [B, C, H, W] -> flat padded [B, C, FLAT] (see module docstring)
    xpad = np.zeros((x.shape[0], C, FLAT), dtype=np.float32)
    arr = xpad[:, :, 2:].reshape(x.shape[0], C, NPR, PW)
    arr[:, :, 1:H + 1, 0:W] = x
    return xpad


def kernel(x: np.ndarray, W: np.ndarray, b: np.ndarray) -> np.ndarray:
    from concourse.bass_utils import run_bass_kernel_spmd

    xpad = _pad_images(np.ascontiguousarray(x, dtype=np.float32))
    wf = _prep_weights(np.asarray(W), np.asarray(b))

    nc = _get_nc()
    in_maps = [{"xb": xpad[c], "wf": wf} for c in range(N_CORES)]
    res = run_bass_kernel_spmd(nc, in_maps, core_ids=list(range(N_CORES)))
    return np.stack([res.results[c]["out"] for c in range(N_CORES)], axis=0)


# revision 31
# speedup vs baseline: 1.1005x; 1.1005x over previous
"""GA3 Conv2d kernel for 8 Trainium2 NeuronCores.

Math: the reference computes, per batch image,
    out[b, co, m] = sum_{j,k} S[m,j,k] * (conv2d(a_k, W[j]) + bias[j])[co]
with a_k = x[:, k::8] (blade-interleaved channels).  Because the sign
combination is linear, it folds into the conv weights:
    V[co*8+m, ci*8+k, kh, kw] = sum_j S[m,j,k] * W[j, co, ci, kh, kw]
    bias_eff[co*8+m]          = sum_{j,k} S[m,j,k] * b[j, co]
so the whole module is ONE dense 3x3 conv with Cin=Cout=128 on
[B, 128, 128, 128].  We shard data-parallel over B across the 8 cores
(1 image per core) and implement the conv as 9 shifted float32r matmuls
per 4-row output block (tap weights stationary [ic=128 x oc=128], pixels
moving, fp32 PSUM accumulation).

Layout: the host pre-pads each image into a flat per-partition buffer
    [pad pair][row: 128 data + pad pair] x 130 padded rows
(pitch 130, zeros at the halo) so every device-side load is a fully
contiguous DMA and tap shifts are pure address offsets.  Input rows
stream in on the SP HWDGE ring in 9 chunks while weights+bias (packed
into one [128, 9*128+1] tensor) and output stores use the ACT ring.
Dummy warm-up matmuls run during the head DMAs to lift the PE HAM clock
gate to 2.4 GHz before the real work starts.
"""

import numpy as np

_TERMS = [
    [(0, 0, 1), (1, 1, 1), (2, 2, 1), (3, 3, 1), (4, 4, -1), (5, 5, -1), (6, 6, -1), (7, 7, -1)],
    [(1, 0, 1), (0, 1, 1), (2, 4, 1), (4, 2, -1), (3, 6, 1), (6, 3, -1), (5, 7, -1), (7, 5, -1)],
    [(2, 0, 1), (0, 2, 1), (1, 4, -1), (4, 1, 1), (3, 5, 1), (5, 3, -1), (6, 7, 1), (7, 6, 1)],
    [(3, 0, 1), (0, 3, 1), (1, 6, -1), (6, 1, 1), (2, 5, -1), (5, 2, 1), (4, 7, -1), (7, 4, -1)],
    [(4, 0, 1), (0, 4, 1), (2, 1, 1), (1, 2, -1), (3, 7, 1), (7, 3, 1), (6, 5, 1), (5, 6, -1)],
    [(5, 0, 1), (0, 5, 1), (3, 2, 1), (2, 3, -1), (1, 7, 1), (7, 1, 1), (4, 6, 1), (6, 4, -1)],
    [(6, 0, 1), (0, 6, 1), (3, 1, 1), (1, 3, -1), (2, 7, -1), (7, 2, -1), (5, 4, 1), (4, 5, -1)],
    [(7, 0, 1), (0, 7, 1), (5, 1, 1), (1, 5, 1), (6, 2, -1), (2, 6, -1), (4, 3, 1), (3, 4, 1)],
]
_S = np.zeros((8, 8, 8), dtype=np.float32)
for _m, _terms in enumerate(_TERMS):
    for _j, _k, _s in _terms:
        _S[_m, _j, _k] = _s

B, CIN, COUT, H, W = 8, 16, 16, 128, 128
C = 8 * CIN  # 128 interleaved channels
N_CORES = 8
STRIP = 16          # output rows per strip (one store DMA)
N_STRIPS = H // STRIP
GROUP = 4           # output rows per PSUM accumulation group (4*128 = 512 free)
PW = W + 2          # padded row pitch in the flat layout
NPR = H + 2         # padded rows (-1 .. 128)
FLAT = 2 + NPR * PW  # flat elems/partition: leading pad pair + 130 rows
WCOLS = 9 * C + 1   # packed weight taps + bias column
N_WARMUP = 8        # HAM warm-up matmuls during the head DMAs

_CACHED_NC = None


def _build_nc():
    import concourse.bass as bass
    import concourse.mybir as mybir
    import concourse.tile as tile
    from concourse import bacc

    f32 = mybir.dt.float32
    f16 = mybir.dt.float16

    nc = bacc.Bacc("TRN2", target_bir_lowering=False, debug=False,
                   enable_asserts=False)

    xb = nc.dram_tensor("xb", [C, FLAT], f16, kind="ExternalInput").ap()
    wf = nc.dram_tensor("wf", [C, WCOLS], f16, kind="ExternalInput").ap()
    out = nc.dram_tensor("out", [C, H, W], f32, kind="ExternalOutput").ap()

    with tile.TileContext(nc) as tc:
        with (
            tc.tile_pool(name="wpool", bufs=1) as wpool,
            tc.tile_pool(name="xpool", bufs=1) as xpool,
            tc.tile_pool(name="pspool", bufs=8, space="PSUM") as pspool,
            tc.tile_pool(name="opool", bufs=3) as opool,
        ):
            xfull = xpool.tile([C, FLAT], f16)
            wtile = wpool.tile([C, WCOLS], f16)

            # weights split so early taps unblock the first matmuls sooner
            nc.scalar.dma_start(out=wtile[:, 0:2 * C], in_=wf[:, 0:2 * C])
            nc.scalar.dma_start(out=wtile[:, 2 * C:WCOLS],
                                in_=wf[:, 2 * C:WCOLS])
            # DVE tensor_scalar needs an fp32 scalar operand — up-convert the
            # packed fp16 bias column once
            btile = wpool.tile([C, 1], f32)
            nc.vector.tensor_copy(out=btile[:, :], in_=wtile[:, 9 * C:WCOLS])

            # input chunks (contiguous flat ranges at padded-row boundaries);
            # first chunk halved so the first PSUM group starts early.  Only
            # the first three are issued at the head — the rest are emitted
            # interleaved with the strips so head DMAs don't collide on the
            # 8 shared DMA-completion semaphore lanes.
            bounds = [0, 6, 9, 25, 41, 57, 73, 89, 105, 121, NPR]

            def emit_chunk(c):
                a, b = bounds[c], bounds[c + 1]
                lo = 0 if a == 0 else 2 + PW * a
                hi = 2 + PW * b if b < NPR else FLAT
                nc.sync.dma_start(out=xfull[:, lo:hi], in_=xb[:, lo:hi])

            for c in range(5):
                emit_chunk(c)

            # ---- conv: 32 PSUM groups x 9 shifted matmuls
            for s in range(N_STRIPS):
                if 2 <= s <= 6:
                    emit_chunk(s + 3)   # stays ~2 strips ahead of consumption
                obuf = opool.tile([C, STRIP * W], f32)
                for g in range(STRIP // GROUP):
                    ps = pspool.tile([C, GROUP * W], f32)
                    ta = 0
                    for dh in range(3):
                        for dw in range(3):
                            # out rows R..R+3 read padded rows R+dh..R+dh+3
                            # at column shift dw-1; padded row pr's data
                            # starts at flat offset 2 + 130*pr
                            base = 1 + PW * (16 * s + 4 * g + dh) + dw
                            rhs = bass.AP(xfull.tensor, xfull.offset + base,
                                          [xfull.ap[0], [PW, GROUP], [1, W]])
                            nc.tensor.matmul(
                                ps[:, :],
                                lhsT=wtile[:, ta * C:(ta + 1) * C],
                                rhs=rhs,
                                start=(ta == 0),
                                stop=(ta == 8),
                            )
                            ta += 1
                    nc.vector.tensor_scalar_add(
                        out=obuf[:, g * GROUP * W:(g + 1) * GROUP * W],
                        in0=ps[:, :],
                        scalar1=btile[:, 0:1],
                    )
                if s < N_STRIPS - 1:
                    nc.scalar.dma_start(
                        out=out[:, s * STRIP:(s + 1) * STRIP, :],
                        in_=obuf[:, :])
                else:
                    # split the final store so the tail barrier waits on a
                    # small last transfer
                    for g in range(STRIP // GROUP):
                        nc.scalar.dma_start(
                            out=out[:, s * STRIP + g * GROUP:
                                    s * STRIP + (g + 1) * GROUP, :],
                            in_=obuf[:, g * GROUP * W:(g + 1) * GROUP * W])

    nc.compile()
    return nc


def _get_nc():
    global _CACHED_NC
    if _CACHED_NC is None:
        _CACHED_NC = _build_nc()
    return _CACHED_NC


def _prep_weights(Wfull: np.ndarray, b: np.ndarray):
    # wf[ic, tap*128 + oc] with ic = ci*8+k, oc = co*8+m, tap = kh*3+kw;
    # final column (index 9*128) holds bias_eff[oc] indexed by partition.
    V = np.einsum("mjk,jcihw->ikhwcm", _S.astype(np.float64),
                  Wfull.astype(np.float64))          # [ci,k,kh,kw,co,m]
    V = V.reshape(C, 9 * C)
    bias = np.einsum("mjk,jc->cm", _S.astype(np.float64),
                     b.astype(np.float64)).reshape(C, 1)
    wf = np.concatenate([V, bias], axis=1)
    return np.ascontiguousarray(wf, dtype=np.float16)


def _pad_images(x: np.ndarray) -> np.ndarray:
    # [B, C, H, W] -> flat padded [B, C, FLAT] (see module docstring)
    xpad = np.zeros((x.shape[0], C, FLAT), dtype=np.float16)
    arr = xpad[:, :, 2:].reshape(x.shape[0], C, NPR, PW)
    arr[:, :, 1:H + 1, 0:W] = x
    return xpad


def kernel(x: np.ndarray, W: np.ndarray, b: np.ndarray) -> np.ndarray:
    from concourse.bass_utils import run_bass_kernel_spmd

    xpad = _pad_images(np.ascontiguousarray(x, dtype=np.float32))
    wf = _prep_weights(np.asarray(W), np.asarray(b))

    nc = _get_nc()
    in_maps = [{"xb": xpad[c], "wf": wf} for c in range(N_CORES)]
    res = run_bass_kernel_spmd(nc, in_maps, core_ids=list(range(N_CORES)))
    return np.stack([res.results[c]["out"] for c in range(N_CORES)], axis=0)


# revision 35
# speedup vs baseline: 1.1148x; 1.0130x over previous
"""GA3 Conv2d kernel for 8 Trainium2 NeuronCores.

Math: the reference computes, per batch image,
    out[b, co, m] = sum_{j,k} S[m,j,k] * (conv2d(a_k, W[j]) + bias[j])[co]
with a_k = x[:, k::8] (blade-interleaved channels).  Because the sign
combination is linear, it folds into the conv weights:
    V[co*8+m, ci*8+k, kh, kw] = sum_j S[m,j,k] * W[j, co, ci, kh, kw]
    bias_eff[co*8+m]          = sum_{j,k} S[m,j,k] * b[j, co]
so the whole module is ONE dense 3x3 conv with Cin=Cout=128 on
[B, 128, 128, 128].  We shard data-parallel over B across the 8 cores
(1 image per core) and implement the conv as 9 shifted float32r matmuls
per 4-row output block (tap weights stationary [ic=128 x oc=128], pixels
moving, fp32 PSUM accumulation).

Layout: the host pre-pads each image into a flat per-partition buffer
    [pad pair][row: 128 data + pad pair] x 130 padded rows
(pitch 130, zeros at the halo) so every device-side load is a fully
contiguous DMA and tap shifts are pure address offsets.  Input rows
stream in on the SP HWDGE ring in 9 chunks while weights+bias (packed
into one [128, 9*128+1] tensor) and output stores use the ACT ring.
Dummy warm-up matmuls run during the head DMAs to lift the PE HAM clock
gate to 2.4 GHz before the real work starts.
"""

import numpy as np

_TERMS = [
    [(0, 0, 1), (1, 1, 1), (2, 2, 1), (3, 3, 1), (4, 4, -1), (5, 5, -1), (6, 6, -1), (7, 7, -1)],
    [(1, 0, 1), (0, 1, 1), (2, 4, 1), (4, 2, -1), (3, 6, 1), (6, 3, -1), (5, 7, -1), (7, 5, -1)],
    [(2, 0, 1), (0, 2, 1), (1, 4, -1), (4, 1, 1), (3, 5, 1), (5, 3, -1), (6, 7, 1), (7, 6, 1)],
    [(3, 0, 1), (0, 3, 1), (1, 6, -1), (6, 1, 1), (2, 5, -1), (5, 2, 1), (4, 7, -1), (7, 4, -1)],
    [(4, 0, 1), (0, 4, 1), (2, 1, 1), (1, 2, -1), (3, 7, 1), (7, 3, 1), (6, 5, 1), (5, 6, -1)],
    [(5, 0, 1), (0, 5, 1), (3, 2, 1), (2, 3, -1), (1, 7, 1), (7, 1, 1), (4, 6, 1), (6, 4, -1)],
    [(6, 0, 1), (0, 6, 1), (3, 1, 1), (1, 3, -1), (2, 7, -1), (7, 2, -1), (5, 4, 1), (4, 5, -1)],
    [(7, 0, 1), (0, 7, 1), (5, 1, 1), (1, 5, 1), (6, 2, -1), (2, 6, -1), (4, 3, 1), (3, 4, 1)],
]
_S = np.zeros((8, 8, 8), dtype=np.float32)
for _m, _terms in enumerate(_TERMS):
    for _j, _k, _s in _terms:
        _S[_m, _j, _k] = _s

B, CIN, COUT, H, W = 8, 16, 16, 128, 128
C = 8 * CIN  # 128 interleaved channels
N_CORES = 8
STRIP = 16          # output rows per strip (one store DMA)
N_STRIPS = H // STRIP
GROUP = 4           # output rows per PSUM accumulation group (4*128 = 512 free)
PW = W + 2          # padded row pitch in the flat layout
NPR = H + 2         # padded rows (-1 .. 128)
FLAT = 2 + NPR * PW  # flat elems/partition: leading pad pair + 130 rows
WCOLS = 9 * C + 1   # packed weight taps + bias column
N_WARMUP = 8        # HAM warm-up matmuls during the head DMAs

_CACHED_NC = None


def _build_nc():
    import concourse.bass as bass
    import concourse.mybir as mybir
    import concourse.tile as tile
    from concourse import bacc

    f32 = mybir.dt.float32
    f16 = mybir.dt.float16

    nc = bacc.Bacc("TRN2", target_bir_lowering=False, debug=False,
                   enable_asserts=False)

    xb = nc.dram_tensor("xb", [C, FLAT], f16, kind="ExternalInput").ap()
    wf = nc.dram_tensor("wf", [C, WCOLS], f16, kind="ExternalInput").ap()
    out = nc.dram_tensor("out", [C, H, W], f32, kind="ExternalOutput").ap()

    with tile.TileContext(nc) as tc:
        with (
            tc.tile_pool(name="wpool", bufs=1) as wpool,
            tc.tile_pool(name="xpool", bufs=1) as xpool,
            tc.tile_pool(name="pspool", bufs=8, space="PSUM") as pspool,
            tc.tile_pool(name="opool", bufs=3) as opool,
        ):
            xfull = xpool.tile([C, FLAT], f16)
            wtile = wpool.tile([C, WCOLS], f16)

            # All loads ride the SP ring in FIFO order (wf first), so each
            # DMA's completion semaphore lands right after its own transfer
            # instead of draining behind cross-ring traffic.  Stores get the
            # ACT ring to themselves.
            nc.sync.dma_start(out=wtile[:, :], in_=wf[:, :])
            # DVE tensor_scalar needs an fp32 scalar operand — up-convert the
            # packed fp16 bias column once
            btile = wpool.tile([C, 1], f32)
            nc.vector.tensor_copy(out=btile[:, :], in_=wtile[:, 9 * C:WCOLS])

            # HAM warm-up: dep-free junk matmuls on a memset scratch tile run
            # right after the preamble, lifting the PE clock gate to 2.4 GHz
            # before the data-gated real matmuls begin.
            wmsrc = wpool.tile([C, 512], f16)
            nc.vector.memset(wmsrc[:, :], 0.0)
            wmps = pspool.tile([C, GROUP * W], f32, tag="ps")
            for _ in range(5):
                nc.tensor.matmul(wmps[:, :], lhsT=wmsrc[:, 0:C],
                                 rhs=wmsrc[:, 0:512], start=True, stop=True,
                                 skip_group_check=True)

            # input chunks (contiguous flat ranges at padded-row boundaries);
            # first chunk halved so the first PSUM group starts early.  Only
            # the first three are issued at the head — the rest are emitted
            # interleaved with the strips so head DMAs don't collide on the
            # 8 shared DMA-completion semaphore lanes.
            bounds = [0, 6, 9, 25, 41, 57, 73, 89, 105, 121, NPR]

            def emit_chunk(c):
                a, b = bounds[c], bounds[c + 1]
                lo = 0 if a == 0 else 2 + PW * a
                hi = 2 + PW * b if b < NPR else FLAT
                nc.sync.dma_start(out=xfull[:, lo:hi], in_=xb[:, lo:hi])

            for c in range(4):
                emit_chunk(c)

            # ---- conv: 32 PSUM groups x 9 shifted matmuls
            for s in range(N_STRIPS):
                if 1 <= s <= 6:
                    emit_chunk(s + 3)   # stays ~2 strips ahead of consumption
                obuf = opool.tile([C, STRIP * W], f32)
                for g in range(STRIP // GROUP):
                    ps = pspool.tile([C, GROUP * W], f32)
                    ta = 0
                    for dh in range(3):
                        for dw in range(3):
                            # out rows R..R+3 read padded rows R+dh..R+dh+3
                            # at column shift dw-1; padded row pr's data
                            # starts at flat offset 2 + 130*pr
                            base = 1 + PW * (16 * s + 4 * g + dh) + dw
                            rhs = bass.AP(xfull.tensor, xfull.offset + base,
                                          [xfull.ap[0], [PW, GROUP], [1, W]])
                            nc.tensor.matmul(
                                ps[:, :],
                                lhsT=wtile[:, ta * C:(ta + 1) * C],
                                rhs=rhs,
                                start=(ta == 0),
                                stop=(ta == 8),
                            )
                            ta += 1
                    nc.vector.tensor_scalar_add(
                        out=obuf[:, g * GROUP * W:(g + 1) * GROUP * W],
                        in0=ps[:, :],
                        scalar1=btile[:, 0:1],
                    )
                if s < N_STRIPS - 1:
                    nc.scalar.dma_start(
                        out=out[:, s * STRIP:(s + 1) * STRIP, :],
                        in_=obuf[:, :])
                else:
                    # split the final store so the tail barrier waits on a
                    # small last transfer
                    for g in range(STRIP // GROUP):
                        nc.scalar.dma_start(
                            out=out[:, s * STRIP + g * GROUP:
                                    s * STRIP + (g + 1) * GROUP, :],
                            in_=obuf[:, g * GROUP * W:(g + 1) * GROUP * W])

    nc.compile()
    return nc


def _get_nc():
    global _CACHED_NC
    if _CACHED_NC is None:
        _CACHED_NC = _build_nc()
    return _CACHED_NC


def _prep_weights(Wfull: np.ndarray, b: np.ndarray):
    # wf[ic, tap*128 + oc] with ic = ci*8+k, oc = co*8+m, tap = kh*3+kw;
    # final column (index 9*128) holds bias_eff[oc] indexed by partition.
    V = np.einsum("mjk,jcihw->ikhwcm", _S.astype(np.float64),
                  Wfull.astype(np.float64))          # [ci,k,kh,kw,co,m]
    V = V.reshape(C, 9 * C)
    bias = np.einsum("mjk,jc->cm", _S.astype(np.float64),
                     b.astype(np.float64)).reshape(C, 1)
    wf = np.concatenate([V, bias], axis=1)
    return np.ascontiguousarray(wf, dtype=np.float16)


def _pad_images(x: np.ndarray) -> np.ndarray:
    # [B, C, H, W] -> flat padded [B, C, FLAT] (see module docstring)
    xpad = np.zeros((x.shape[0], C, FLAT), dtype=np.float16)
    arr = xpad[:, :, 2:].reshape(x.shape[0], C, NPR, PW)
    arr[:, :, 1:H + 1, 0:W] = x
    return xpad


def kernel(x: np.ndarray, W: np.ndarray, b: np.ndarray) -> np.ndarray:
    from concourse.bass_utils import run_bass_kernel_spmd

    xpad = _pad_images(np.ascontiguousarray(x, dtype=np.float32))
    wf = _prep_weights(np.asarray(W), np.asarray(b))

    nc = _get_nc()
    in_maps = [{"xb": xpad[c], "wf": wf} for c in range(N_CORES)]
    res = run_bass_kernel_spmd(nc, in_maps, core_ids=list(range(N_CORES)))
    return np.stack([res.results[c]["out"] for c in range(N_CORES)], axis=0)


# revision 36
# speedup vs baseline: 1.1420x; 1.0244x over previous
"""GA3 Conv2d kernel for 8 Trainium2 NeuronCores.

Math: the reference computes, per batch image,
    out[b, co, m] = sum_{j,k} S[m,j,k] * (conv2d(a_k, W[j]) + bias[j])[co]
with a_k = x[:, k::8] (blade-interleaved channels).  Because the sign
combination is linear, it folds into the conv weights:
    V[co*8+m, ci*8+k, kh, kw] = sum_j S[m,j,k] * W[j, co, ci, kh, kw]
    bias_eff[co*8+m]          = sum_{j,k} S[m,j,k] * b[j, co]
so the whole module is ONE dense 3x3 conv with Cin=Cout=128 on
[B, 128, 128, 128].  We shard data-parallel over B across the 8 cores
(1 image per core) and implement the conv as 9 shifted float32r matmuls
per 4-row output block (tap weights stationary [ic=128 x oc=128], pixels
moving, fp32 PSUM accumulation).

Layout: the host pre-pads each image into a flat per-partition buffer
    [pad pair][row: 128 data + pad pair] x 130 padded rows
(pitch 130, zeros at the halo) so every device-side load is a fully
contiguous DMA and tap shifts are pure address offsets.  Input rows
stream in on the SP HWDGE ring in 9 chunks while weights+bias (packed
into one [128, 9*128+1] tensor) and output stores use the ACT ring.
Dummy warm-up matmuls run during the head DMAs to lift the PE HAM clock
gate to 2.4 GHz before the real work starts.
"""

import numpy as np

_TERMS = [
    [(0, 0, 1), (1, 1, 1), (2, 2, 1), (3, 3, 1), (4, 4, -1), (5, 5, -1), (6, 6, -1), (7, 7, -1)],
    [(1, 0, 1), (0, 1, 1), (2, 4, 1), (4, 2, -1), (3, 6, 1), (6, 3, -1), (5, 7, -1), (7, 5, -1)],
    [(2, 0, 1), (0, 2, 1), (1, 4, -1), (4, 1, 1), (3, 5, 1), (5, 3, -1), (6, 7, 1), (7, 6, 1)],
    [(3, 0, 1), (0, 3, 1), (1, 6, -1), (6, 1, 1), (2, 5, -1), (5, 2, 1), (4, 7, -1), (7, 4, -1)],
    [(4, 0, 1), (0, 4, 1), (2, 1, 1), (1, 2, -1), (3, 7, 1), (7, 3, 1), (6, 5, 1), (5, 6, -1)],
    [(5, 0, 1), (0, 5, 1), (3, 2, 1), (2, 3, -1), (1, 7, 1), (7, 1, 1), (4, 6, 1), (6, 4, -1)],
    [(6, 0, 1), (0, 6, 1), (3, 1, 1), (1, 3, -1), (2, 7, -1), (7, 2, -1), (5, 4, 1), (4, 5, -1)],
    [(7, 0, 1), (0, 7, 1), (5, 1, 1), (1, 5, 1), (6, 2, -1), (2, 6, -1), (4, 3, 1), (3, 4, 1)],
]
_S = np.zeros((8, 8, 8), dtype=np.float32)
for _m, _terms in enumerate(_TERMS):
    for _j, _k, _s in _terms:
        _S[_m, _j, _k] = _s

B, CIN, COUT, H, W = 8, 16, 16, 128, 128
C = 8 * CIN  # 128 interleaved channels
N_CORES = 8
STRIP = 16          # output rows per strip (one store DMA)
N_STRIPS = H // STRIP
GROUP = 4           # output rows per PSUM accumulation group (4*128 = 512 free)
PW = W + 2          # padded row pitch in the flat layout
NPR = H + 2         # padded rows (-1 .. 128)
FLAT = 2 + NPR * PW  # flat elems/partition: leading pad pair + 130 rows
WCOLS = 9 * C + 1   # packed weight taps + bias column
N_WARMUP = 8        # HAM warm-up matmuls during the head DMAs

_CACHED_NC = None


def _build_nc():
    import concourse.bass as bass
    import concourse.mybir as mybir
    import concourse.tile as tile
    from concourse import bacc

    f32 = mybir.dt.float32
    f16 = mybir.dt.float16

    nc = bacc.Bacc("TRN2", target_bir_lowering=False, debug=False,
                   enable_asserts=False)

    xb = nc.dram_tensor("xb", [C, FLAT], f16, kind="ExternalInput").ap()
    wf = nc.dram_tensor("wf", [C, WCOLS], f16, kind="ExternalInput").ap()
    out = nc.dram_tensor("out", [C, H, W], f32, kind="ExternalOutput").ap()

    with tile.TileContext(nc) as tc:
        with (
            tc.tile_pool(name="wpool", bufs=1) as wpool,
            tc.tile_pool(name="xpool", bufs=1) as xpool,
            tc.tile_pool(name="pspool", bufs=8, space="PSUM") as pspool,
            tc.tile_pool(name="opool", bufs=3) as opool,
        ):
            xfull = xpool.tile([C, FLAT], f16)
            wtile = wpool.tile([C, WCOLS], f16)

            # All loads ride the SP ring in FIFO order (wf first), so each
            # DMA's completion semaphore lands right after its own transfer
            # instead of draining behind cross-ring traffic.  Stores get the
            # ACT ring to themselves.
            nc.sync.dma_start(out=wtile[:, :], in_=wf[:, :])
            # DVE tensor_scalar needs an fp32 scalar operand — up-convert the
            # packed fp16 bias column once
            btile = wpool.tile([C, 1], f32)
            nc.vector.tensor_copy(out=btile[:, :], in_=wtile[:, 9 * C:WCOLS])

            # HAM warm-up: dep-free junk matmuls on a memset scratch tile run
            # right after the preamble, lifting the PE clock gate to 2.4 GHz
            # before the data-gated real matmuls begin.
            wmsrc = wpool.tile([C, 512], f16)
            nc.vector.memset(wmsrc[:, :], 0.0)
            wmps = pspool.tile([C, GROUP * W], f32, tag="ps")
            for _ in range(8):
                nc.tensor.matmul(wmps[:, :], lhsT=wmsrc[:, 0:C],
                                 rhs=wmsrc[:, 0:512], start=True, stop=True,
                                 skip_group_check=True)

            # input chunks (contiguous flat ranges at padded-row boundaries);
            # first chunk halved so the first PSUM group starts early.  Only
            # the first three are issued at the head — the rest are emitted
            # interleaved with the strips so head DMAs don't collide on the
            # 8 shared DMA-completion semaphore lanes.
            bounds = [0, 6, 9, 25, 41, 57, 73, 89, 105, 121, NPR]

            def emit_chunk(c):
                a, b = bounds[c], bounds[c + 1]
                lo = 0 if a == 0 else 2 + PW * a
                hi = 2 + PW * b if b < NPR else FLAT
                nc.sync.dma_start(out=xfull[:, lo:hi], in_=xb[:, lo:hi])

            for c in range(4):
                emit_chunk(c)

            # ---- conv: 32 PSUM groups x 9 shifted matmuls
            for s in range(N_STRIPS):
                if 1 <= s <= 6:
                    emit_chunk(s + 3)   # stays ~2 strips ahead of consumption
                obuf = opool.tile([C, STRIP * W], f32)
                for g in range(STRIP // GROUP):
                    ps = pspool.tile([C, GROUP * W], f32)
                    ta = 0
                    for dh in range(3):
                        for dw in range(3):
                            # out rows R..R+3 read padded rows R+dh..R+dh+3
                            # at column shift dw-1; padded row pr's data
                            # starts at flat offset 2 + 130*pr
                            base = 1 + PW * (16 * s + 4 * g + dh) + dw
                            rhs = bass.AP(xfull.tensor, xfull.offset + base,
                                          [xfull.ap[0], [PW, GROUP], [1, W]])
                            nc.tensor.matmul(
                                ps[:, :],
                                lhsT=wtile[:, ta * C:(ta + 1) * C],
                                rhs=rhs,
                                start=(ta == 0),
                                stop=(ta == 8),
                            )
                            ta += 1
                    nc.vector.tensor_scalar_add(
                        out=obuf[:, g * GROUP * W:(g + 1) * GROUP * W],
                        in0=ps[:, :],
                        scalar1=btile[:, 0:1],
                    )
                if s < N_STRIPS - 1:
                    nc.scalar.dma_start(
                        out=out[:, s * STRIP:(s + 1) * STRIP, :],
                        in_=obuf[:, :])
                else:
                    # split the final store so the tail barrier waits on a
                    # small last transfer
                    for g in range(STRIP // GROUP):
                        nc.scalar.dma_start(
                            out=out[:, s * STRIP + g * GROUP:
                                    s * STRIP + (g + 1) * GROUP, :],
                            in_=obuf[:, g * GROUP * W:(g + 1) * GROUP * W])

    nc.compile()
    return nc


def _get_nc():
    global _CACHED_NC
    if _CACHED_NC is None:
        _CACHED_NC = _build_nc()
    return _CACHED_NC


def _prep_weights(Wfull: np.ndarray, b: np.ndarray):
    # wf[ic, tap*128 + oc] with ic = ci*8+k, oc = co*8+m, tap = kh*3+kw;
    # final column (index 9*128) holds bias_eff[oc] indexed by partition.
    V = np.einsum("mjk,jcihw->ikhwcm", _S.astype(np.float64),
                  Wfull.astype(np.float64))          # [ci,k,kh,kw,co,m]
    V = V.reshape(C, 9 * C)
    bias = np.einsum("mjk,jc->cm", _S.astype(np.float64),
                     b.astype(np.float64)).reshape(C, 1)
    wf = np.concatenate([V, bias], axis=1)
    return np.ascontiguousarray(wf, dtype=np.float16)


def _pad_images(x: np.ndarray) -> np.ndarray:
    # [B, C, H, W] -> flat padded [B, C, FLAT] (see module docstring)
    xpad = np.zeros((x.shape[0], C, FLAT), dtype=np.float16)
    arr = xpad[:, :, 2:].reshape(x.shape[0], C, NPR, PW)
    arr[:, :, 1:H + 1, 0:W] = x
    return xpad


def kernel(x: np.ndarray, W: np.ndarray, b: np.ndarray) -> np.ndarray:
    from concourse.bass_utils import run_bass_kernel_spmd

    xpad = _pad_images(np.ascontiguousarray(x, dtype=np.float32))
    wf = _prep_weights(np.asarray(W), np.asarray(b))

    nc = _get_nc()
    in_maps = [{"xb": xpad[c], "wf": wf} for c in range(N_CORES)]
    res = run_bass_kernel_spmd(nc, in_maps, core_ids=list(range(N_CORES)))
    return np.stack([res.results[c]["out"] for c in range(N_CORES)], axis=0)


# revision 38
# speedup vs baseline: 1.1568x; 1.0130x over previous
"""GA3 Conv2d kernel for 8 Trainium2 NeuronCores.

Math: the reference computes, per batch image,
    out[b, co, m] = sum_{j,k} S[m,j,k] * (conv2d(a_k, W[j]) + bias[j])[co]
with a_k = x[:, k::8] (blade-interleaved channels).  Because the sign
combination is linear, it folds into the conv weights:
    V[co*8+m, ci*8+k, kh, kw] = sum_j S[m,j,k] * W[j, co, ci, kh, kw]
    bias_eff[co*8+m]          = sum_{j,k} S[m,j,k] * b[j, co]
so the whole module is ONE dense 3x3 conv with Cin=Cout=128 on
[B, 128, 128, 128].  We shard data-parallel over B across the 8 cores
(1 image per core) and implement the conv as 9 shifted fp16 matmuls per
4-row output block (tap weights stationary [ic=128 x oc=128], pixels
moving, fp32 PSUM accumulation; fp16 operands keep LDWEIGHTS fully
hidden behind the 512-column matmul stream and halve input DMA bytes;
measured rel err ~3e-4).

Layout: the host pre-pads each image into a flat per-partition buffer
    [pad pair][row: 128 data + pad pair] x 130 padded rows
(pitch 130, zeros at the halo) so every device-side load is a fully
contiguous DMA and tap shifts are pure address offsets.  All loads ride
the SP HWDGE ring in FIFO order (completion semaphores land right after
their own transfer); stores get the ACT ring.  Dep-free warm-up matmuls
on a memset scratch tile run during the head DMAs so the PE HAM clock
gate is already at 2.4 GHz when the data-gated real matmuls start.
"""

import numpy as np

_TERMS = [
    [(0, 0, 1), (1, 1, 1), (2, 2, 1), (3, 3, 1), (4, 4, -1), (5, 5, -1), (6, 6, -1), (7, 7, -1)],
    [(1, 0, 1), (0, 1, 1), (2, 4, 1), (4, 2, -1), (3, 6, 1), (6, 3, -1), (5, 7, -1), (7, 5, -1)],
    [(2, 0, 1), (0, 2, 1), (1, 4, -1), (4, 1, 1), (3, 5, 1), (5, 3, -1), (6, 7, 1), (7, 6, 1)],
    [(3, 0, 1), (0, 3, 1), (1, 6, -1), (6, 1, 1), (2, 5, -1), (5, 2, 1), (4, 7, -1), (7, 4, -1)],
    [(4, 0, 1), (0, 4, 1), (2, 1, 1), (1, 2, -1), (3, 7, 1), (7, 3, 1), (6, 5, 1), (5, 6, -1)],
    [(5, 0, 1), (0, 5, 1), (3, 2, 1), (2, 3, -1), (1, 7, 1), (7, 1, 1), (4, 6, 1), (6, 4, -1)],
    [(6, 0, 1), (0, 6, 1), (3, 1, 1), (1, 3, -1), (2, 7, -1), (7, 2, -1), (5, 4, 1), (4, 5, -1)],
    [(7, 0, 1), (0, 7, 1), (5, 1, 1), (1, 5, 1), (6, 2, -1), (2, 6, -1), (4, 3, 1), (3, 4, 1)],
]
_S = np.zeros((8, 8, 8), dtype=np.float32)
for _m, _terms in enumerate(_TERMS):
    for _j, _k, _s in _terms:
        _S[_m, _j, _k] = _s

B, CIN, COUT, H, W = 8, 16, 16, 128, 128
C = 8 * CIN  # 128 interleaved channels
N_CORES = 8
STRIP = 16          # output rows per strip (one store DMA)
N_STRIPS = H // STRIP
GROUP = 4           # output rows per PSUM accumulation group (4*128 = 512 free)
PW = W + 2          # padded row pitch in the flat layout
NPR = H + 2         # padded rows (-1 .. 128)
FLAT = 2 + NPR * PW  # flat elems/partition: leading pad pair + 130 rows
WCOLS = 9 * C + 1   # packed weight taps + bias column
N_WARMUP = 8        # HAM warm-up matmuls during the head DMAs

_CACHED_NC = None


def _build_nc():
    import concourse.bass as bass
    import concourse.mybir as mybir
    import concourse.tile as tile
    from concourse import bacc

    f32 = mybir.dt.float32
    f16 = mybir.dt.float16

    nc = bacc.Bacc("TRN2", target_bir_lowering=False, debug=False,
                   enable_asserts=False)

    xb = nc.dram_tensor("xb", [C, FLAT], f16, kind="ExternalInput").ap()
    wf = nc.dram_tensor("wf", [C, WCOLS], f16, kind="ExternalInput").ap()
    out = nc.dram_tensor("out", [C, H, W], f32, kind="ExternalOutput").ap()

    with tile.TileContext(nc) as tc:
        with (
            tc.tile_pool(name="wpool", bufs=1) as wpool,
            tc.tile_pool(name="xpool", bufs=1) as xpool,
            tc.tile_pool(name="pspool", bufs=8, space="PSUM") as pspool,
            tc.tile_pool(name="opool", bufs=3) as opool,
        ):
            xfull = xpool.tile([C, FLAT], f16)
            wtile = wpool.tile([C, WCOLS], f16)

            # All loads ride the SP ring in FIFO order (wf first), so each
            # DMA's completion semaphore lands right after its own transfer
            # instead of draining behind cross-ring traffic.  Stores get the
            # ACT ring to themselves.
            nc.sync.dma_start(out=wtile[:, :], in_=wf[:, :])
            # DVE tensor_scalar needs an fp32 scalar operand — up-convert the
            # packed fp16 bias column once
            btile = wpool.tile([C, 1], f32)
            nc.vector.tensor_copy(out=btile[:, :], in_=wtile[:, 9 * C:WCOLS])

            # HAM warm-up: dep-free junk matmuls on a memset scratch tile run
            # right after the preamble, lifting the PE clock gate to 2.4 GHz
            # before the data-gated real matmuls begin.
            wmsrc = wpool.tile([C, 512], f16)
            nc.vector.memset(wmsrc[:, :], 0.0)
            wmps = pspool.tile([C, GROUP * W], f32, tag="ps")
            for _ in range(N_WARMUP):
                nc.tensor.matmul(wmps[:, :], lhsT=wmsrc[:, 0:C],
                                 rhs=wmsrc[:, 0:512], start=True, stop=True,
                                 skip_group_check=True)

            # input chunks (contiguous flat ranges at padded-row boundaries);
            # first chunk halved so the first PSUM group starts early.  Only
            # the first three are issued at the head — the rest are emitted
            # interleaved with the strips so head DMAs don't collide on the
            # 8 shared DMA-completion semaphore lanes.
            bounds = [0, 6, 9, 25, 41, 57, 73, 89, 105, 121, NPR]

            def emit_chunk(c):
                a, b = bounds[c], bounds[c + 1]
                lo = 0 if a == 0 else 2 + PW * a
                hi = 2 + PW * b if b < NPR else FLAT
                nc.sync.dma_start(out=xfull[:, lo:hi], in_=xb[:, lo:hi])

            for c in range(4):
                emit_chunk(c)

            # ---- conv: 32 PSUM groups x 9 shifted matmuls
            for s in range(N_STRIPS):
                if 1 <= s <= 6:
                    emit_chunk(s + 3)   # stays ~2 strips ahead of consumption
                obuf = opool.tile([C, STRIP * W], f32)
                for g in range(STRIP // GROUP):
                    ps = pspool.tile([C, GROUP * W], f32)
                    ta = 0
                    for dh in range(3):
                        for dw in range(3):
                            # out rows R..R+3 read padded rows R+dh..R+dh+3
                            # at column shift dw-1; padded row pr's data
                            # starts at flat offset 2 + 130*pr
                            base = 1 + PW * (16 * s + 4 * g + dh) + dw
                            rhs = bass.AP(xfull.tensor, xfull.offset + base,
                                          [xfull.ap[0], [PW, GROUP], [1, W]])
                            nc.tensor.matmul(
                                ps[:, :],
                                lhsT=wtile[:, ta * C:(ta + 1) * C],
                                rhs=rhs,
                                start=(ta == 0),
                                stop=(ta == 8),
                            )
                            ta += 1
                    nc.vector.tensor_scalar_add(
                        out=obuf[:, g * GROUP * W:(g + 1) * GROUP * W],
                        in0=ps[:, :],
                        scalar1=btile[:, 0:1],
                    )
                if s < N_STRIPS - 1:
                    nc.scalar.dma_start(
                        out=out[:, s * STRIP:(s + 1) * STRIP, :],
                        in_=obuf[:, :])
                else:
                    # split the final store so the tail barrier waits on a
                    # small last transfer
                    for g in range(STRIP // GROUP):
                        nc.scalar.dma_start(
                            out=out[:, s * STRIP + g * GROUP:
                                    s * STRIP + (g + 1) * GROUP, :],
                            in_=obuf[:, g * GROUP * W:(g + 1) * GROUP * W])

    nc.compile()
    return nc


def _get_nc():
    global _CACHED_NC
    if _CACHED_NC is None:
        _CACHED_NC = _build_nc()
    return _CACHED_NC


def _prep_weights(Wfull: np.ndarray, b: np.ndarray):
    # wf[ic, tap*128 + oc] with ic = ci*8+k, oc = co*8+m, tap = kh*3+kw;
    # final column (index 9*128) holds bias_eff[oc] indexed by partition.
    V = np.einsum("mjk,jcihw->ikhwcm", _S.astype(np.float64),
                  Wfull.astype(np.float64))          # [ci,k,kh,kw,co,m]
    V = V.reshape(C, 9 * C)
    bias = np.einsum("mjk,jc->cm", _S.astype(np.float64),
                     b.astype(np.float64)).reshape(C, 1)
    wf = np.concatenate([V, bias], axis=1)
    return np.ascontiguousarray(wf, dtype=np.float16)


def _pad_images(x: np.ndarray) -> np.ndarray:
    # [B, C, H, W] -> flat padded [B, C, FLAT] (see module docstring)
    xpad = np.zeros((x.shape[0], C, FLAT), dtype=np.float16)
    arr = xpad[:, :, 2:].reshape(x.shape[0], C, NPR, PW)
    arr[:, :, 1:H + 1, 0:W] = x
    return xpad


def kernel(x: np.ndarray, W: np.ndarray, b: np.ndarray) -> np.ndarray:
    from concourse.bass_utils import run_bass_kernel_spmd

    xpad = _pad_images(np.ascontiguousarray(x, dtype=np.float32))
    wf = _prep_weights(np.asarray(W), np.asarray(b))

    nc = _get_nc()
    in_maps = [{"xb": xpad[c], "wf": wf} for c in range(N_CORES)]
    res = run_bass_kernel_spmd(nc, in_maps, core_ids=list(range(N_CORES)))
    return np.stack([res.results[c]["out"] for c in range(N_CORES)], axis=0)
